# revision 1
# baseline (speedup 1.0000x reference)
"""DenseCL contrastive loss on 8 Trainium2 NeuronCores (Bass/Tile).

Strategy: data-parallel over batch B=128 -> 16 batches/core for the dense
heads; the global (pooled) heads are sharded over the hidden dim (256/core)
for all 128 batches, stitched with a tiny g-AllGather + z-AllReduce that
overlap the dense matmuls.

Per core (T-layouts, contraction dims live on partitions):
  - dense head per branch:  H1T = W1^T @ X^T (bf16), ZT = W2^T @ H1T,
    l2norm over De via gpsimd partition_all_reduce -> f1T/f2T [128, 784]
  - global head: g = mean_HW(X) AllGathered to [C, 128], per-core hidden
    slice of the MLP, partial z AllReduced -> qgT/kgT [128, 128] replicated
  - sim_b = f1T_b^T @ f2T_b -> [49,49]; DVE max/max_index -> pos + argmax
  - matchedT gathered from f2T columns via gpsimd ap_gather, AllGathered
  - row-block of InfoNCE logits: f1T^T @ matchedT_all, exp on ACT with
    accumulated row sums, log -> lse; loss partials reduced to scalars
Host sums 4 partial scalars per core into the final loss.
"""

import sys

sys.path.insert(0, "/opt/trn_rl_repo")

import numpy as np
import ml_dtypes

import concourse.bacc as bacc
import concourse.mybir as mybir
import concourse.bass_isa as bass_isa
import concourse.tile as tile
from concourse.bass_utils import run_bass_kernel_spmd

dt = mybir.dt
AF = mybir.ActivationFunctionType

N_CORES = 8
B, H, W, C = 128, 7, 7, 2048
DH, DE = 2048, 128
HW = H * W                      # 49
BL = B // N_CORES               # 16 batches per core
PIX = BL * HW                   # 784 pixels per core
GPIX = B * HW                   # 6272 global rows
TAU_INV = 5.0
KC = C // 128                   # 16 contraction chunks
MC = DH // 128                  # 16 hidden chunks
DSL = DH // N_CORES             # 256: global-head hidden slice per core
MSL = DSL // 128                # 2 chunks of the slice

_NC = None


def _build(timing=False, stop=None):
    # timing=True builds a single-core cost-model variant: collectives are
    # skipped and gathered results are read from the local bounce buffer.
    # stop: for cost-model bisection - truncate after a named phase.
    _ph = ["bk", "glob", "bq", "sim", "gather", "rhs", "logits", "all"]
    lim = _ph.index(stop) if stop else len(_ph) - 1

    def go(p):
        return _ph.index(p) <= lim

    nc = bacc.Bacc("TRN2", target_bir_lowering=False, debug=False,
                   num_devices=N_CORES)

    def inp(name, shape, d=dt.bfloat16):
        return nc.dram_tensor(name, shape, d, kind="ExternalInput").ap()

    xqT = inp("xqT", [C, PIX])
    xkT = inp("xkT", [C, PIX])
    wd1 = inp("wd1", [C, DH])
    wmd1 = inp("wmd1", [C, DH])
    wd2 = inp("wd2", [DH, DE])
    wmd2 = inp("wmd2", [DH, DE])
    wg1s = inp("wg1s", [C, DSL])        # per-core hidden slice of Wg1
    wmg1s = inp("wmg1s", [C, DSL])
    wg2s = inp("wg2s", [DSL, DE])
    wmg2s = inp("wmg2s", [DSL, DE])
    ball = inp("ball", [128, 41], dt.float32)
    eye = inp("eye", [64, 64], dt.float32)
    out = nc.dram_tensor("partials", [1, 8], dt.float32,
                         kind="ExternalOutput").ap()

    with tile.TileContext(nc) as tc:
        with (
            tc.tile_pool(name="pers", bufs=1) as pers,
            tc.tile_pool(name="wz", bufs=2) as wz,
            tc.tile_pool(name="work", bufs=2) as work,
            tc.tile_pool(name="dram", bufs=1, space="DRAM") as dram,
        ):
            # ---- constants / biases (single load) ----
            eyesb = pers.tile([64, 64], dt.float32, name="eyesb")
            nc.sync.dma_start(out=eyesb[:], in_=eye[:])
            ballsb = pers.tile([128, 41], dt.float32, name="ballsb")
            nc.sync.dma_start(out=ballsb[:], in_=ball[:])
            _bcols = {"bd1": (0, MC), "mbd1": (16, MC), "bg1s": (32, MSL),
                      "mbg1s": (34, MSL), "bd2": (36, 1), "bg2": (37, 1),
                      "mbd2": (38, 1), "mbg2": (39, 1)}
            biases = {nm: ballsb[:, c0:c0 + w_]
                      for nm, (c0, w_) in _bcols.items()}
            addsb = ballsb[0:BL, 40:41]

            def load_xt(x_dram, nm):
                ts = []
                for k in range(KC):
                    t = pers.tile([128, PIX], dt.bfloat16, name=f"{nm}{k}")
                    nc.sync.dma_start(
                        out=t[:, 0:PIX // 2],
                        in_=x_dram[k * 128:(k + 1) * 128, 0:PIX // 2])
                    nc.sync.dma_start(
                        out=t[:, PIX // 2:],
                        in_=x_dram[k * 128:(k + 1) * 128, PIX // 2:])
                    ts.append(t)
                return ts

            def norm_cols(z, n, nm, outs):
                """l2-normalize columns of z [128, n] (De on partitions)."""
                sq = work.tile([128, n], dt.float32, tag=f"sq{n}",
                               name=f"sq_{nm}")
                nc.vector.tensor_mul(sq[:], z[:], z[:])
                ssr = work.tile([128, n], dt.float32, tag=f"ssr{n}",
                                name=f"ssr_{nm}")
                nc.gpsimd.partition_all_reduce(ssr[:], sq[:], 128,
                                               bass_isa.ReduceOp.add)
                nc.vector.tensor_scalar_max(ssr[:], ssr[:], 1e-12)
                srt = work.tile([128, n], dt.float32, tag=f"srt{n}",
                                name=f"srt_{nm}")
                nc.scalar.activation(srt[:], ssr[:], AF.Sqrt)
                rr = work.tile([128, n], dt.float32, tag=f"rr{n}",
                               name=f"rr_{nm}")
                nc.vector.reciprocal(rr[:], srt[:])
                for o in outs:
                    nc.vector.tensor_mul(o, z[:], rr[:])

            def gmean(xts, nm):
                """mean over HW -> [128, KC*BL] bf16 (c-chunk x batch)."""
                gsum = work.tile([128, BL * KC], dt.float32, tag="gsum",
                                 name=f"gsum_{nm}")
                for k in range(KC):
                    nc.vector.tensor_reduce(
                        gsum[:, k * BL:(k + 1) * BL],
                        xts[k][:].rearrange("p (b w) -> p b w", w=HW),
                        axis=mybir.AxisListType.X, op=mybir.AluOpType.add)
                gt = work.tile([128, BL * KC], dt.bfloat16, tag="gt",
                               name=f"gt_{nm}")
                nc.vector.tensor_scalar_mul(gt[:], gsum[:], 1.0 / HW)
                return gt

            with tc.tile_pool(name="ps", bufs=2, space="PSUM") as ps:

                def dense_branch(xts, w1_dram, b1, w2_dram, b2, nm,
                                 extra_dma):
                    """2-layer dense head -> un-normalized ZT [128, 784]."""
                    w2sb = wz.tile([128, MC * 128], dt.bfloat16, tag="w2sb",
                                   name=f"w2_{nm}")
                    w2v = w2sb[:].rearrange("p (k m) -> p k m", m=DE)
                    w2s = w2_dram[:].rearrange("(k p) m -> p k m", p=128)
                    for q4 in range(0, MC, 4):
                        nc.sync.dma_start(out=w2v[:, q4:q4 + 4],
                                          in_=w2s[:, q4:q4 + 4])

                    ztp = ps.tile([128, PIX], dt.float32, tag="ztp", bufs=1,
                                  name=f"ztp_{nm}")
                    for m in range(MC):
                        wcol = wz.tile([128, KC * 128], dt.bfloat16,
                                       tag="wcold", bufs=3,
                                       name=f"wcold_{nm}{m}")
                        wv = wcol[:].rearrange("p (k m) -> p k m", m=128)
                        sv = w1_dram[:, m * 128:(m + 1) * 128].rearrange(
                            "(k p) m -> p k m", p=128)
                        for q4 in range(0, KC, 4):
                            nc.sync.dma_start(out=wv[:, q4:q4 + 4],
                                              in_=sv[:, q4:q4 + 4])
                        if m < len(extra_dma):
                            dst, src = extra_dma[m]
                            nc.sync.dma_start(out=dst, in_=src)
                        h1p = ps.tile([128, PIX], dt.float32, tag="bigp",
                                      name=f"h1p_{nm}{m}")
                        for k in range(KC):
                            lhs = wcol[:, k * 128:(k + 1) * 128]
                            nc.tensor.matmul(h1p[:, 0:512], lhs,
                                             xts[k][:, 0:512],
                                             start=(k == 0),
                                             stop=(k == KC - 1))
                            nc.tensor.matmul(h1p[:, 512:PIX], lhs,
                                             xts[k][:, 512:PIX],
                                             start=(k == 0),
                                             stop=(k == KC - 1))
                        h1sb = work.tile([128, PIX], dt.bfloat16, tag="h1sb",
                                         bufs=3, name=f"h1_{nm}{m}")
                        nc.scalar.activation(h1sb[:], h1p[:], AF.Relu,
                                             bias=b1[:, m:m + 1])
                        lhs2 = w2sb[:, m * 128:(m + 1) * 128]
                        nc.tensor.matmul(ztp[:, 0:512], lhs2, h1sb[:, 0:512],
                                         start=(m == 0), stop=(m == MC - 1))
                        nc.tensor.matmul(ztp[:, 512:PIX], lhs2,
                                         h1sb[:, 512:PIX],
                                         start=(m == 0), stop=(m == MC - 1))
                    zt = work.tile([128, PIX], dt.float32, tag="zt",
                                   name=f"zt_{nm}")
                    nc.vector.tensor_scalar_add(zt[:], ztp[:], b2)
                    return zt

                # ---- load X, momentum branch first ----
                xkts = load_xt(xkT, "xk")
                xq_dmas = []
                xqts = []
                for k in range(KC):
                    t = pers.tile([128, PIX], dt.bfloat16, name=f"xq{k}")
                    xqts.append(t)
                    xq_dmas.append((t[:], xqT[k * 128:(k + 1) * 128, :]))

                ztk = dense_branch(xkts, wmd1, biases["mbd1"], wmd2,
                                   biases["mbd2"], "k", xq_dmas)
                f2tb = pers.tile([128, PIX], dt.bfloat16, name="f2tb")
                f2tf = pers.tile([128, PIX], dt.float32, name="f2tf")
                norm_cols(ztk, PIX, "f2", [f2tb[:], f2tf[:]])

                # ---- g means + AllGather (overlaps q dense branch) ----
                gtk = gmean(xkts, "k")
                gtq = gmean(xqts, "q")
                gagin = dram.tile([128, 2 * BL * KC], dt.bfloat16,
                                  name="gagin")
                gagout = dram.tile([128 * N_CORES, 2 * BL * KC], dt.bfloat16,
                                   addr_space="Shared", name="gagout")
                nc.sync.dma_start(out=gagin[:, 0:BL * KC], in_=gtk[:])
                nc.sync.dma_start(out=gagin[:, BL * KC:], in_=gtq[:])
                if not timing:
                    nc.gpsimd.collective_compute(
                        "AllGather", mybir.AluOpType.bypass,
                        replica_groups=[list(range(N_CORES))],
                        ins=[gagin.opt()], outs=[gagout.opt()])

                if go("glob"):
                    gallk = pers.tile([128, KC * B], dt.bfloat16,
                                      name="gallk")
                    gallq = pers.tile([128, KC * B], dt.bfloat16,
                                      name="gallq")
                    for r in range(N_CORES):
                        src = (gagin if timing
                               else gagout[r * 128:(r + 1) * 128, :])
                        nc.sync.dma_start(
                            out=gallk[:].rearrange(
                                "p (k b) -> p k b",
                                b=B)[:, :, r * BL:(r + 1) * BL],
                            in_=src[:, 0:BL * KC].rearrange(
                                "p (k b) -> p k b", b=BL))
                        nc.sync.dma_start(
                            out=gallq[:].rearrange(
                                "p (k b) -> p k b",
                                b=B)[:, :, r * BL:(r + 1) * BL],
                            in_=src[:, BL * KC:].rearrange(
                                "p (k b) -> p k b", b=BL))

                    # per-core hidden slice of the global MLP, all batches
                    def global_head(gall, w1s_dram, b1s, w2s_dram, nm):
                        w1sb = wz.tile([128, KC * DSL], dt.bfloat16,
                                       tag="wg1sb", name=f"wg1s_{nm}")
                        wv = w1sb[:].rearrange("p (k m) -> p k m", m=DSL)
                        sv = w1s_dram[:].rearrange("(k p) m -> p k m", p=128)
                        nc.sync.dma_start(out=wv[:, 0:KC // 2],
                                          in_=sv[:, 0:KC // 2])
                        nc.sync.dma_start(out=wv[:, KC // 2:],
                                          in_=sv[:, KC // 2:])
                        w2ssb = wz.tile([128, MSL * DE], dt.bfloat16,
                                        tag="wg2ssb", name=f"wg2s_{nm}")
                        nc.sync.dma_start(
                            out=w2ssb[:].rearrange("p (k m) -> p k m", m=DE),
                            in_=w2s_dram[:].rearrange("(k p) m -> p k m",
                                                      p=128))
                        hgs = work.tile([128, MSL * B], dt.bfloat16,
                                        tag="hgs", name=f"hgs_{nm}")
                        for ml in range(MSL):
                            hp = ps.tile([128, B], dt.float32, tag="smallp",
                                         name=f"hp_{nm}{ml}")
                            for k in range(KC):
                                nc.tensor.matmul(
                                    hp[:],
                                    w1sb[:, k * DSL + ml * 128:
                                         k * DSL + (ml + 1) * 128],
                                    gall[:, k * B:(k + 1) * B],
                                    start=(k == 0), stop=(k == KC - 1))
                            nc.scalar.activation(
                                hgs[:, ml * B:(ml + 1) * B], hp[:], AF.Relu,
                                bias=b1s[:, ml:ml + 1])
                        zp = ps.tile([128, B], dt.float32, tag="smallp",
                                     name=f"zp_{nm}")
                        for ml in range(MSL):
                            nc.tensor.matmul(zp[:],
                                             w2ssb[:, ml * DE:(ml + 1) * DE],
                                             hgs[:, ml * B:(ml + 1) * B],
                                             start=(ml == 0),
                                             stop=(ml == MSL - 1))
                        return zp

                    zpk = global_head(gallk, wmg1s, biases["mbg1s"], wmg2s,
                                      "k")
                    zpq = global_head(gallq, wg1s, biases["bg1s"], wg2s, "q")
                    zpart = work.tile([128, 2 * B], dt.float32, tag="zpart",
                                      name="zpart")
                    nc.vector.tensor_copy(zpart[:, 0:B], zpk[:])
                    nc.vector.tensor_copy(zpart[:, B:2 * B], zpq[:])
                    arin = dram.tile([128, 2 * B], dt.float32, name="arin")
                    arout = dram.tile([128, 2 * B], dt.float32,
                                      addr_space="Shared", name="arout")
                    nc.sync.dma_start(out=arin[:], in_=zpart[:])
                    if not timing:
                        nc.gpsimd.collective_compute(
                            "AllReduce", mybir.AluOpType.add,
                            replica_groups=[list(range(N_CORES))],
                            ins=[arin.opt()], outs=[arout.opt()])
                    zall = work.tile([128, 2 * B], dt.float32, tag="zall",
                                     name="zall")
                    nc.sync.dma_start(out=zall[:],
                                      in_=arin[:] if timing else arout[:])
                    zgk = work.tile([128, B], dt.float32, tag="zgk",
                                    name="zgk")
                    nc.vector.tensor_scalar_add(zgk[:], zall[:, 0:B],
                                                biases["mbg2"])
                    zgq = work.tile([128, B], dt.float32, tag="zgq",
                                    name="zgq")
                    nc.vector.tensor_scalar_add(zgq[:], zall[:, B:2 * B],
                                                biases["bg2"])
                    kgb = pers.tile([128, B], dt.bfloat16, name="kgb")
                    kgf = pers.tile([128, B], dt.float32, name="kgf")
                    norm_cols(zgk, B, "kg", [kgb[:], kgf[:]])
                    qgb = pers.tile([128, B], dt.bfloat16, name="qgb")
                    qgf = pers.tile([128, B], dt.float32, name="qgf")
                    norm_cols(zgq, B, "qg", [qgb[:], qgf[:]])

                # ---- query dense branch ----
                if go("bq"):
                    ztq = dense_branch(xqts, wd1, biases["bd1"], wd2,
                                       biases["bd2"], "q", [])
                    f1tb = pers.tile([128, PIX], dt.bfloat16, name="f1tb")
                    norm_cols(ztq, PIX, "f1", [f1tb[:]])

                # ---- per-batch sim + argmax ----
                if go("sim"):
                    maxv = pers.tile([64, BL], dt.float32, name="maxv")
                    nc.vector.memset(maxv[:], 0.0)
                    idxc = pers.tile([64, BL], dt.float32, name="idxc")
                    for b in range(BL):
                        simp = ps.tile([64, HW], dt.float32, tag="smallp",
                                       name=f"simp{b}")
                        nc.tensor.matmul(simp[0:HW, :],
                                         f1tb[:, b * HW:(b + 1) * HW],
                                         f2tb[:, b * HW:(b + 1) * HW],
                                         start=True, stop=True)
                        simsb = work.tile([64, HW], dt.float32, tag="simsb",
                                          name=f"sims{b}")
                        nc.vector.tensor_copy(simsb[0:HW, :], simp[0:HW, :])
                        mx8 = work.tile([64, 8], dt.float32, tag="mx8",
                                        name=f"mx{b}")
                        mi8 = work.tile([64, 8], dt.uint16, tag="mi8",
                                        name=f"mi{b}")
                        nc.vector.max(mx8[0:HW, :], simsb[0:HW, :])
                        nc.vector.max_index(mi8[0:HW, :], mx8[0:HW, :],
                                            simsb[0:HW, :])
                        nc.vector.tensor_copy(maxv[0:HW, b:b + 1],
                                              mx8[0:HW, 0:1])
                        nc.vector.tensor_copy(idxc[0:HW, b:b + 1],
                                              mi8[0:HW, 0:1])

                # ---- wrapped gather indices, gather, AllGather ----
                if go("gather"):
                    tpp = ps.tile([BL, 64], dt.float32, tag="smallp",
                                  name="tpp")
                    nc.tensor.transpose(tpp[0:BL, 0:HW], idxc[0:HW, 0:BL],
                                        eyesb[0:HW, 0:HW])
                    idxf = work.tile([BL, HW], dt.float32, tag="idxf",
                                     name="idxf")
                    nc.vector.tensor_scalar_add(idxf[:], tpp[0:BL, 0:HW],
                                                addsb)
                    idxw = work.tile([BL, HW], dt.int16, tag="idxw",
                                     name="idxw")
                    nc.vector.tensor_copy(idxw[:], idxf[:])
                    idxr = pers.tile([128, HW], dt.int16, name="idxr")
                    for g in range(8):
                        nc.sync.dma_start(out=idxr[g * 16:(g + 1) * 16, :],
                                          in_=idxw[:])
                    mtf = pers.tile([128, PIX], dt.float32, name="mtf")
                    nc.gpsimd.ap_gather(mtf[:], f2tf[:], idxr[:],
                                        channels=128, num_elems=PIX, d=1,
                                        num_idxs=PIX)
                    mtb = pers.tile([128, PIX], dt.bfloat16, name="mtb")
                    nc.vector.tensor_copy(mtb[:], mtf[:])
                    ag1in = dram.tile([128, PIX], dt.bfloat16, name="ag1in")
                    ag1out = dram.tile([128 * N_CORES, PIX], dt.bfloat16,
                                       addr_space="Shared", name="ag1out")
                    for q4 in range(4):
                        nc.sync.dma_start(
                            out=ag1in[:, q4 * PIX // 4:(q4 + 1) * PIX // 4],
                            in_=mtb[:, q4 * PIX // 4:(q4 + 1) * PIX // 4])
                    if not timing:
                        nc.gpsimd.collective_compute(
                            "AllGather", mybir.AluOpType.bypass,
                            replica_groups=[list(range(N_CORES))],
                            ins=[ag1in.opt()], outs=[ag1out.opt()])

            # ---- logits phase: fresh PSUM pool (wide exp chunks) ----
            with tc.tile_pool(name="ps2", bufs=2, space="PSUM") as ps2:
                if go("rhs"):
                    rhs = pers.tile([128, GPIX], dt.bfloat16, name="rhs")
                    for r in range(N_CORES):
                        src1 = (ag1in[:] if timing
                                else ag1out[r * 128:(r + 1) * 128, :])
                        nc.sync.dma_start(out=rhs[:, r * PIX:(r + 1) * PIX],
                                          in_=src1)

                if go("logits"):
                    chunks = []
                    c0 = 0
                    while c0 < GPIX:
                        csz = min(2048, GPIX - c0)
                        chunks.append((c0, csz))
                        c0 += csz
                    ncat = len(chunks)  # 4

                    rsums = pers.tile([128, 8], dt.float32, name="rsums")
                    nc.vector.memset(rsums[:], 1.0)
                    for t in range(7):
                        m0 = t * 128
                        m = min(128, PIX - m0)
                        rs = work.tile([128, 8], dt.float32, tag="rs",
                                       name=f"rs{t}")
                        for ci, (c0, csz) in enumerate(chunks):
                            lp = ps2.tile([128, 2048], dt.float32, tag="lp",
                                          name=f"lp{t}_{ci}")
                            for h0 in range(0, csz, 512):
                                hsz = min(512, csz - h0)
                                nc.tensor.matmul(
                                    lp[0:m, h0:h0 + hsz],
                                    f1tb[:, m0:m0 + m],
                                    rhs[:, c0 + h0:c0 + h0 + hsz],
                                    start=True, stop=True)
                            expsb = work.tile([128, 2048], dt.bfloat16,
                                              tag="expsb",
                                              name=f"ex{t}_{ci}")
                            nc.scalar.activation(expsb[0:m, 0:csz],
                                                 lp[0:m, 0:csz], AF.Exp,
                                                 scale=TAU_INV,
                                                 accum_out=rs[0:m,
                                                              ci:ci + 1])
                        nc.vector.tensor_reduce(rsums[0:m, t:t + 1],
                                                rs[0:m, 0:ncat],
                                                axis=mybir.AxisListType.X,
                                                op=mybir.AluOpType.add)
                    lse = pers.tile([128, 8], dt.float32, name="lse")
                    nc.scalar.activation(lse[:, 0:7], rsums[:, 0:7], AF.Ln)

                if go("all"):
                    # global InfoNCE, replicated over all 128 rows
                    lg = ps2.tile([128, B], dt.float32, tag="lp", name="lg")
                    nc.tensor.matmul(lg[:], qgb[:], kgb[:, 0:B],
                                     start=True, stop=True)
                    expg = work.tile([128, B], dt.bfloat16, tag="expg",
                                     name="expg")
                    eg = work.tile([128, 1], dt.float32, tag="eg", name="eg")
                    nc.scalar.activation(expg[:], lg[:], AF.Exp,
                                         scale=TAU_INV, accum_out=eg[:, 0:1])
                    lseg = work.tile([128, 1], dt.float32, tag="lseg",
                                     name="lseg")
                    nc.scalar.activation(lseg[:], eg[:], AF.Ln)
                    pg = work.tile([128, B], dt.float32, tag="pg", name="pg")
                    nc.vector.tensor_mul(pg[:], qgf[:], kgf[:])

                    partials = pers.tile([1, 8], dt.float32,
                                         name="partials_sb")

                    def psum_scalar(src, n_part, n_free, col, tagn):
                        red = work.tile([n_part, 1], dt.float32,
                                        tag=f"red{tagn}", name=f"red{tagn}")
                        if n_free > 1:
                            nc.vector.tensor_reduce(
                                red[:], src, axis=mybir.AxisListType.X,
                                op=mybir.AluOpType.add)
                        else:
                            nc.vector.tensor_copy(red[:], src)
                        alr = work.tile([n_part, 1], dt.float32,
                                        tag=f"alr{tagn}", name=f"alr{tagn}")
                        nc.gpsimd.partition_all_reduce(
                            alr[:], red[:], n_part, bass_isa.ReduceOp.add)
                        nc.vector.tensor_copy(partials[0:1, col:col + 1],
                                              alr[0:1, 0:1])

                    psum_scalar(lse[:, 0:7], 128, 7, 0, "a")    # sum lse_d
                    psum_scalar(maxv[:, 0:BL], 64, BL, 1, "b")  # sum max sim
                    psum_scalar(lseg[:, 0:1], 128, 1, 2, "c")   # sum lse_g x8
                    psum_scalar(pg[:, 0:B], 128, B, 3, "d")     # sum qg.kg x8

                    nc.sync.dma_start(out=out[:], in_=partials[:])

    nc.compile()
    return nc


def _get_nc():
    global _NC
    if _NC is None:
        _NC = _build()
    return _NC


def _prep_inputs(inputs):
    bf = ml_dtypes.bfloat16
    f32 = np.float32
    wb = {k: np.ascontiguousarray(np.asarray(inputs[k]).astype(bf))
          for k in ("Wd1", "Wd2", "Wg1", "Wg2",
                    "mWd1", "mWd2", "mWg1", "mWg2")}

    def b1(v, mc):
        return np.ascontiguousarray(np.asarray(v, f32).reshape(mc, 128).T)

    def b2(v):
        return np.ascontiguousarray(np.asarray(v, f32).reshape(128, 1))

    ball0 = np.zeros((128, 41), f32)
    ball0[:, 0:MC] = b1(inputs["bd1"], MC)
    ball0[:, 16:16 + MC] = b1(inputs["mbd1"], MC)
    ball0[:, 36] = b2(inputs["bd2"])[:, 0]
    ball0[:, 37] = b2(inputs["bg2"])[:, 0]
    ball0[:, 38] = b2(inputs["mbd2"])[:, 0]
    ball0[:, 39] = b2(inputs["mbg2"])[:, 0]
    ball0[0:BL, 40] = HW * np.arange(BL, dtype=f32)
    common = {
        "wd1": wb["Wd1"], "wmd1": wb["mWd1"],
        "wd2": wb["Wd2"], "wmd2": wb["mWd2"],
        "eye": np.eye(64, dtype=f32),
    }
    fq = np.asarray(inputs["feat_q"], f32).reshape(B, HW, C)
    fk = np.asarray(inputs["feat_k"], f32).reshape(B, HW, C)
    in_maps = []
    for r in range(N_CORES):
        sl = slice(r * BL, (r + 1) * BL)
        hsl = slice(r * DSL, (r + 1) * DSL)
        m = dict(common)
        m["xqT"] = np.ascontiguousarray(fq[sl].reshape(PIX, C).T.astype(bf))
        m["xkT"] = np.ascontiguousarray(fk[sl].reshape(PIX, C).T.astype(bf))
        m["wg1s"] = np.ascontiguousarray(wb["Wg1"][:, hsl])
        m["wmg1s"] = np.ascontiguousarray(wb["mWg1"][:, hsl])
        m["wg2s"] = np.ascontiguousarray(wb["Wg2"][hsl, :])
        m["wmg2s"] = np.ascontiguousarray(wb["mWg2"][hsl, :])
        ballr = ball0.copy()
        ballr[:, 32:32 + MSL] = b1(np.asarray(inputs["bg1"], f32)[hsl], MSL)
        ballr[:, 34:34 + MSL] = b1(np.asarray(inputs["mbg1"], f32)[hsl], MSL)
        m["ball"] = ballr
        in_maps.append(m)
    return in_maps


def _combine(results):
    sld = smd = slg = spg = 0.0
    for r in range(N_CORES):
        p = np.asarray(results[r]["partials"], np.float64).reshape(-1)
        sld += p[0]
        smd += p[1]
        slg += p[2]   # replicated on every core
        spg += p[3]   # replicated on every core
    slg /= N_CORES
    spg /= N_CORES
    l_d = (sld - TAU_INV * smd) / GPIX
    l_g = (slg - TAU_INV * spg) / B
    return np.float32(0.5 * l_g + 0.5 * l_d)


def kernel(**inputs) -> np.ndarray:
    nc = _get_nc()
    in_maps = _prep_inputs(inputs)
    res = run_bass_kernel_spmd(nc, in_maps, list(range(N_CORES)))
    return np.asarray(_combine(res.results))


if __name__ == "__main__":
    import jax
    import reference

    with jax.default_device(jax.devices("cpu")[0]):
        inputs = {k: np.asarray(v)
                  for k, v in reference.setup_inputs().items()}
        exp = np.asarray(reference.reference(**reference.setup_inputs()))
    got = kernel(**inputs)
    print("got", got, "exp", exp, "relerr", abs(got / exp - 1.0))

